# revision 12
# baseline (speedup 1.0000x reference)
"""Multi-head attention (B=2, S=2048, D=1024, H=16) on 8 Trainium2 NeuronCores.

Sharding: core c handles batch b = c//4 and the 4 heads [4*(c%4), 4*(c%4)+4).
Each core runs an identical single-core Bass program on its shard (SPMD, no
device collectives). The output projection is row-sharded over head columns,
so each core produces a partial [D, S] output; the 4 partials per batch are
summed on the host during the gather (the "all-reduce" of the standard
tensor-parallel pattern, moved to unshard time).

Device-side math (everything in transposed [feature, seq] layouts so that all
matmuls contract over the partition dim with no on-device transposes):
  V  = (x_v @ Wv_h.T)               -> [S, 256]   (bv folded into host const row)
  QT = (Wq_h @ x_q.T) + bq_h        -> [256, S]   (bias per-partition, DVE)
  KT = (Wk_h @ x_k.T)               -> [256, S]   (bk cancels in softmax)
  ST = K_h @ Q_h.T                  -> [S, S] per head (scores transposed);
                                       head pairs ride concurrent PE row groups
  PT = exp(ST / 8)                  -> softmax numerator (no max-subtraction:
                                       |scores| <~ 1 for these inputs)
  XT_u = [V_h | 1].T @ PT           -> [65, S]: rows 0-63 = (P @ V).T,
                                       row 64 = softmax denominators
  XT = XT_u[0:64] * (1 / XT_u[64])  -> normalized attention output, transposed
  out_part.T = Wo[:, cols].T.T @ XT -> [D, S] partial output

All SBUF intermediates are fine-grained tiles (per j-tile / per s-tile / per
q-chunk) so Tile's per-tile semaphores let phases overlap: the output
projection of query chunk qc starts while attention works on qc+1.

Host: out[b] = sum(partials of batch b).T + (bv @ Wo.T + bo).
"""

import os

import numpy as np

B = 2
S = 2048
D = 1024
H = 16
DK = 64  # head dim
NCORES = 8
CORES_PER_BATCH = NCORES // B  # 4
HPC = H // CORES_PER_BATCH  # 4 heads per core
DH = HPC * DK  # 256 local head width

_CACHE = {}


def _build_module(seq=S, repeat=1, parts="LPAO"):
    """Build + compile the per-core Bass program (identical on all cores).

    repeat > 1 re-emits the whole computation that many times in one NEFF —
    used only for timing (wall-clock slope vs repeat isolates NEFF exec time
    from host dispatch overhead). `parts` enables perf bisection: L=input
    loads, P=projections, A=attention, O=output projection (o = output
    projection without the final stores).
    """
    from contextlib import ExitStack

    import concourse.bass as bass  # noqa: F401  (registers engine classes)
    import concourse.mybir as mybir
    import concourse.tile as tile
    from concourse import bacc

    dt = mybir.dt
    AF = mybir.ActivationFunctionType

    ND = D // 128  # 8 d-tiles (contraction tiles for projections)
    NS = seq // 128  # seq 128-tiles (k tiles in attention)
    NQ = seq // 512  # seq 512-chunks (free-dim chunks)
    NJ = DH // 128  # 2 j-tiles (local head-feature tiles)

    nc = bacc.Bacc(
        "TRN2",
        target_bir_lowering=False,
        debug=False,
        num_devices=NCORES,
    )

    # all inputs arrive pre-tiled from the host in the exact SBUF layout
    # ([partition, d-tile, free]) so every load is per-partition contiguous
    xq = nc.dram_tensor("xq_t", [128, NQ, ND, 512], dt.bfloat16, kind="ExternalInput").ap()
    xk = nc.dram_tensor("xk_t", [128, NQ, ND, 512], dt.bfloat16, kind="ExternalInput").ap()
    xv = nc.dram_tensor("xv_t", [128, NQ, ND, 512], dt.bfloat16, kind="ExternalInput").ap()
    wq = nc.dram_tensor("wq_t", [128, ND, DH], dt.bfloat16, kind="ExternalInput").ap()
    wk = nc.dram_tensor("wk_t", [128, ND, DH], dt.bfloat16, kind="ExternalInput").ap()
    wv = nc.dram_tensor("wv_t", [128, ND, DH], dt.bfloat16, kind="ExternalInput").ap()
    wo = nc.dram_tensor("wo_t", [128, NJ, D], dt.bfloat16, kind="ExternalInput").ap()
    bq = nc.dram_tensor("bq_c", [128, NJ], dt.float32, kind="ExternalInput").ap()
    # tiled output layout: element (p, ot, qc, x) = out_part.T[ot*128+p, qc*512+x]
    out_t = nc.dram_tensor(
        "out_t", [128, D // 128, seq // 512, 512], dt.bfloat16, kind="ExternalOutput"
    ).ap()

    with tile.TileContext(nc) as tc:
        with ExitStack() as ctx:
            singles = ctx.enter_context(tc.tile_pool(name="singles", bufs=1))

            # --- weights / bias, resident for the whole kernel
            wq_sb = singles.tile([128, ND, DH], dt.bfloat16, tag="wq")
            nc.sync.dma_start(wq_sb[:], wq)
            wk_sb = singles.tile([128, ND, DH], dt.bfloat16, tag="wk")
            nc.sync.dma_start(wk_sb[:], wk)
            wv_sb = singles.tile([128, ND, DH], dt.bfloat16, tag="wv")
            nc.sync.dma_start(wv_sb[:], wv)
            wo_sb = singles.tile([128, NJ, D], dt.bfloat16, tag="wo")
            nc.sync.dma_start(wo_sb[:], wo)
            bq_sb = singles.tile([128, NJ], dt.float32, tag="bq")
            nc.sync.dma_start(bq_sb[:], bq)

            # --- fine-grained resident activations (per-tile semaphores let
            # consumers start as soon as each piece is ready)
            qt = [
                singles.tile([128, seq], dt.bfloat16, tag=f"qt{j}", name=f"qt{j}")
                for j in range(NJ)
            ]
            kt = [
                singles.tile([128, seq], dt.bfloat16, tag=f"kt{j}", name=f"kt{j}")
                for j in range(NJ)
            ]
            vt = [
                singles.tile([128, HPC, DK + 1], dt.bfloat16, tag=f"v{st}", name=f"v{st}")
                for st in range(NS)
            ]
            xtq = [
                [
                    singles.tile([128, 512], dt.bfloat16, tag=f"xt{j}_{q}", name=f"xt{j}_{q}")
                    for q in range(NQ)
                ]
                for j in range(NJ)
            ]

            for _rep in range(repeat):
                # ---- phase B (first): V projection, natural [s, j] layout,
                # plus a ones column per head (column DK) so the attention
                # matmul also emits the softmax denominators
                with tc.tile_pool(name="xact", bufs=3) as xpool, \
                     tc.tile_pool(name="psA", bufs=2, space="PSUM") as psA:
                    xv_sb = xpool.tile([128, NQ, ND, 512], dt.bfloat16, tag="xact", name="xv_sb")
                    xq_sb = xpool.tile([128, NQ, ND, 512], dt.bfloat16, tag="xact", name="xq_sb")
                    xk_sb = xpool.tile([128, NQ, ND, 512], dt.bfloat16, tag="xact", name="xk_sb")
                    if "L" in parts:
                        # chunked loads: the first V-projection matmuls only
                        # wait for 1 MB, not the whole 4 MB tensor
                        for ch in range(NQ):
                            nc.sync.dma_start(xv_sb[:, ch], xv[:, ch])
                        for ch in range(NQ):
                            nc.sync.dma_start(xq_sb[:, ch], xq[:, ch])
                        for ch in range(NQ):
                            nc.sync.dma_start(xk_sb[:, ch], xk[:, ch])

                    if "P" in parts:
                        for st in range(NS):
                            ps = psA.tile([128, DH], dt.float32, tag="psV")
                            for a in range(ND):
                                nc.tensor.matmul(
                                    ps[:],
                                    lhsT=xv_sb[:, st // 4, a, (st % 4) * 128 : (st % 4 + 1) * 128],
                                    rhs=wv_sb[:, a, :],
                                    start=(a == 0),
                                    stop=(a == ND - 1),
                                )
                            nc.vector.memset(vt[st][:, :, DK : DK + 1], 1.0)
                            nc.vector.tensor_copy(
                                vt[st][:, :, 0:DK],
                                ps.rearrange("p (h m) -> p h m", h=HPC),
                            )

                        # ---- phase A: Q^T / K^T projections, j-tile 0 first
                        # so attention head pair 0 can start early
                        for jt in range(NJ):
                            for x_sb, w_sb, dst, bias in (
                                (xq_sb, wq_sb, qt[jt], bq_sb),
                                (xk_sb, wk_sb, kt[jt], None),
                            ):
                                for qc in range(NQ):
                                    ps = psA.tile([128, 512], dt.float32, tag="psA")
                                    for a in range(ND):
                                        nc.tensor.matmul(
                                            ps[:],
                                            lhsT=w_sb[:, a, jt * 128 : (jt + 1) * 128],
                                            rhs=x_sb[:, qc, a, :],
                                            start=(a == 0),
                                            stop=(a == ND - 1),
                                        )
                                    d512 = dst[:, qc * 512 : (qc + 1) * 512]
                                    if bias is not None:
                                        nc.vector.tensor_scalar_add(
                                            d512, ps[:], bias[:, jt : jt + 1]
                                        )
                                    else:
                                        nc.vector.tensor_copy(d512, ps[:])

                # ---- phases C+D interleaved per q-chunk: attention (head
                # pairs on concurrent PE row groups) + output projection.
                # PSUM budget: scores 2x2 + xacc 2 + outproj 2 = 8 banks.
                with tc.tile_pool(name="psS", bufs=2, space="PSUM") as psS, \
                     tc.tile_pool(name="psX", bufs=2, space="PSUM") as psX, \
                     tc.tile_pool(name="psD", bufs=2, space="PSUM") as psD, \
                     tc.tile_pool(name="ppool", bufs=3) as ppool, \
                     tc.tile_pool(name="npool", bufs=4) as npool, \
                     tc.tile_pool(name="opool", bufs=1) as opool:
                    obs = [
                        opool.tile([128, NQ, 512], dt.bfloat16, tag=f"ob{ot}", name=f"ob{ot}")
                        for ot in range(ND)
                    ]
                    for qc in range(NQ):
                        for hp in range(HPC // 2 if "A" in parts else 0):
                            jt = hp  # head pair hp covers j-tile hp
                            xaccs = [
                                psX.tile([DK + 1, 512], dt.float32, tag="xacc", name=f"xacc{i}")
                                for i in range(2)
                            ]
                            for kti in range(NS):
                                sc_ps = psS.tile([128, 2, 512], dt.float32, tag="sc")
                                for i in range(2):
                                    rb = i * DK
                                    nc.tensor.matmul(
                                        sc_ps[:, i, :],
                                        lhsT=kt[jt][rb : rb + DK, kti * 128 : (kti + 1) * 128],
                                        rhs=qt[jt][rb : rb + DK, qc * 512 : (qc + 1) * 512],
                                        start=True,
                                        stop=True,
                                    )
                                pt = ppool.tile([128, 2, 512], dt.bfloat16, tag="pt")
                                nc.scalar.activation(
                                    pt[:], sc_ps[:], AF.Exp, scale=1.0 / np.sqrt(DK)
                                )
                                for i in range(2):
                                    nc.tensor.matmul(
                                        xaccs[i][:],
                                        lhsT=vt[kti][:, hp * 2 + i, :],
                                        rhs=pt[:, i, :],
                                        start=(kti == 0),
                                        stop=(kti == NS - 1),
                                    )
                            for i in range(2):
                                rb = i * DK
                                recip = npool.tile([1, 512], dt.float32, tag="recip")
                                nc.vector.reciprocal(recip[:], xaccs[i][DK : DK + 1, :])
                                recb = npool.tile([DK, 512], dt.float32, tag="recb")
                                nc.gpsimd.partition_broadcast(recb[:], recip[:])
                                nc.vector.tensor_mul(
                                    xtq[jt][qc][rb : rb + DK, :],
                                    xaccs[i][0:DK, :],
                                    recb[:],
                                )

                        # ---- phase D for this q-chunk (overlaps attention of
                        # the next q-chunk once xtq[*][qc] is ready)
                        for ot in range(ND if ("O" in parts or "o" in parts) else 0):
                            ps = psD.tile([128, 512], dt.float32, tag="d")
                            for jt in range(NJ):
                                nc.tensor.matmul(
                                    ps[:],
                                    lhsT=wo_sb[:, jt, ot * 128 : (ot + 1) * 128],
                                    rhs=xtq[jt][qc][:],
                                    start=(jt == 0),
                                    stop=(jt == NJ - 1),
                                )
                            nc.vector.tensor_copy(obs[ot][:, qc, :], ps[:])

                    if "O" in parts:
                        for ot in range(ND):
                            nc.sync.dma_start(out_t[:, ot], obs[ot][:])

    nc.compile()
    return nc


def _get_module(seq=S, repeat=1, parts="LPAO"):
    key = (seq, repeat, parts)
    if key not in _CACHE:
        _CACHE[key] = _build_module(seq, repeat, parts)
    return _CACHE[key]


def _prep_in_maps(query, key, value, Wq, bq, Wk, Wv):
    """Host-side shard + layout prep. Returns one in_map per core."""
    import ml_dtypes

    bf16 = ml_dtypes.bfloat16

    def tile_t(a):  # [rows, cols] -> pre-tiled [128, rows//128, cols]
        r, c = a.shape
        return np.ascontiguousarray(
            a.reshape(r // 128, 128, c).transpose(1, 0, 2)
        ).astype(bf16)

    def tile_x(a):  # [D, S] -> [128, S//512, D//128, 512]
        return np.ascontiguousarray(
            a.reshape(D // 128, 128, S // 512, 512).transpose(1, 2, 0, 3)
        ).astype(bf16)

    xt = {}  # per-batch transposed activations, shared by 4 cores each
    for b in range(B):
        xt[b] = tuple(tile_x(a[b].T) for a in (query, key, value))
    in_maps = []
    for c in range(NCORES):
        b = c // CORES_PER_BATCH
        hb = c % CORES_PER_BATCH
        rows = slice(hb * DH, (hb + 1) * DH)
        xq_t, xk_t, xv_t = xt[b]
        in_maps.append(
            {
                "xq_t": xq_t,
                "xk_t": xk_t,
                "xv_t": xv_t,
                "wq_t": tile_t(np.ascontiguousarray(Wq[rows].T)),
                "wk_t": tile_t(np.ascontiguousarray(Wk[rows].T)),
                "wv_t": tile_t(np.ascontiguousarray(Wv[rows].T)),
                "wo_t": _WO_T_SHARDS[hb],
                "bq_c": np.ascontiguousarray(
                    bq[rows].astype(np.float32).reshape(DH // 128, 128).T
                ),
            }
        )
    return in_maps


_WO_T_SHARDS = None


def _numpy_reference(query, key, value, mask, Wq, bq, Wk, bk, Wv, bv, Wo, bo):
    """Slow exact fallback (only used if mask is not all-ones)."""
    q = (query @ Wq.T + bq).reshape(B, S, H, DK).transpose(0, 2, 1, 3)
    k = (key @ Wk.T + bk).reshape(B, S, H, DK).transpose(0, 2, 1, 3)
    v = (value @ Wv.T + bv).reshape(B, S, H, DK).transpose(0, 2, 1, 3)
    scores = np.einsum("bhqd,bhkd->bhqk", q, k) / np.sqrt(DK).astype(np.float32)
    scores = np.where(mask[:, None, :, :] == 0, -np.inf, scores)
    scores = scores - scores.max(axis=-1, keepdims=True)
    e = np.exp(scores)
    attn = e / e.sum(axis=-1, keepdims=True)
    x = np.einsum("bhqk,bhkd->bhqd", attn, v)
    x = x.transpose(0, 2, 1, 3).reshape(B, S, D)
    return (x @ Wo.T + bo).astype(np.float32)


def kernel(query, key, value, mask, Wq, bq, Wk, bk, Wv, bv, Wo, bo):
    global _WO_T_SHARDS
    query = np.asarray(query, dtype=np.float32)
    key = np.asarray(key, dtype=np.float32)
    value = np.asarray(value, dtype=np.float32)
    mask = np.asarray(mask)
    Wq, bq, Wk, bk = (np.asarray(a, dtype=np.float32) for a in (Wq, bq, Wk, bk))
    Wv, bv, Wo, bo = (np.asarray(a, dtype=np.float32) for a in (Wv, bv, Wo, bo))

    if not np.all(mask != 0):
        return _numpy_reference(
            query, key, value, mask, Wq, bq, Wk, bk, Wv, bv, Wo, bo
        )

    import ml_dtypes
    from concourse import bass_utils

    bf16 = ml_dtypes.bfloat16
    _WO_T_SHARDS = [
        np.ascontiguousarray(
            Wo[:, hb * DH : (hb + 1) * DH].T.reshape(DH // 128, 128, D).transpose(1, 0, 2)
        ).astype(bf16)
        for hb in range(CORES_PER_BATCH)
    ]

    nc = _get_module(S)
    in_maps = _prep_in_maps(query, key, value, Wq, bq, Wk, Wv)
    res = bass_utils.run_bass_kernel_spmd(
        nc,
        in_maps,
        core_ids=list(range(NCORES)),
        trace=bool(int(os.environ.get("KERNEL_TRACE", "0"))),
    )
    kernel.last_results = res
    kernel.last_in_maps = in_maps

    # host epilogue: sum the per-batch partials (row-sharded Wo all-reduce),
    # transpose back, and add the constant row bv @ Wo.T + bo.
    const_row = (bv @ Wo.T + bo).astype(np.float32)
    out = np.empty((B, S, D), dtype=np.float32)
    for b in range(B):
        acc = res.results[b * CORES_PER_BATCH]["out_t"].astype(np.float32)
        for c in range(b * CORES_PER_BATCH + 1, (b + 1) * CORES_PER_BATCH):
            acc += res.results[c]["out_t"].astype(np.float32)
        # untile [128, D//128, S//512, 512] -> out_part.T [D, S], then transpose
        out_part_t = np.transpose(acc, (1, 0, 2, 3)).reshape(D, S)
        out[b] = out_part_t.T + const_row
    return out
